# revision 34
# baseline (speedup 1.0000x reference)
"""Causal dot-product attention on 8 Trainium2 NeuronCores.

Problem: q,k,v [16, 2048, 128] fp32, causal softmax(q k^T / sqrt(128)) v.
Sharding: heads (N=16) split across 8 cores, 2 heads per core; no cross-core
communication.

Per-core kernel design (per head):
  - Q and K are transposed to [F, T] layout via PE transposes (the matmul
    contraction must sit on the partition dim), V stays natural [T, F] with an
    extra all-ones column appended (provides softmax row-sums for free via the
    attention matmul itself).
  - Scores are computed transposed, scoresT[s, q] = kT_j.T @ qT, one 512-wide
    q-block at a time, accumulated per 4 k-tiles into a [128, 2048] PSUM tile,
    masked on the diagonal band, exp'd on the scalar engine PSUM->SBUF.
  - out[q, f+1] accumulates expT_ij.T @ [v_j | 1] over j in PSUM; column 128 is
    the softmax denominator. Final normalize = per-partition reciprocal + mul.
"""

import numpy as np

import concourse.bass as bass
import concourse.mybir as mybir
import concourse.tile as tile
from concourse import bacc
from concourse.bass import ts
from concourse.bass_utils import run_bass_kernel_spmd
from concourse.masks import make_identity

N, T, F = 16, 2048, 128
N_CORES = 8
H = N // N_CORES  # heads per core
P = 128
NT = T // P  # 16 k/q tiles per head
BLK = 4  # q-tiles per block (512 q columns)
NBLK = NT // BLK
SCALE = 1.0 / float(np.sqrt(F))
NEG = -1.0e9

F32 = mybir.dt.float32
F32R = mybir.dt.float32r  # TF32-like PE mode: 1 cycle/row at N>=256 (fp32 is 4)
BF16 = mybir.dt.bfloat16


def build(masked: bool):
    nc = bacc.Bacc("TRN2", target_bir_lowering=False, debug=False, num_devices=N_CORES)
    q = nc.dram_tensor("q", [H, T, F], F32, kind="ExternalInput")
    k = nc.dram_tensor("k", [H, T, F], F32, kind="ExternalInput")
    v = nc.dram_tensor("v", [H, T, F], F32, kind="ExternalInput")
    out = nc.dram_tensor("out", [H, T, F], F32, kind="ExternalOutput")

    with tile.TileContext(nc) as tc:
        _attention(tc, out, q, k, v, masked)
    nc.compile()
    return nc


def _attention(tc, out, q, k, v, masked: bool):
    from contextlib import ExitStack

    nc = tc.nc
    ctx = ExitStack()
    consts = ctx.enter_context(tc.tile_pool(name="consts", bufs=1))
    nat_pool = ctx.enter_context(tc.tile_pool(name="nat", bufs=4))
    big_pool = ctx.enter_context(tc.tile_pool(name="big", bufs=2))
    vpool = ctx.enter_context(tc.tile_pool(name="vpool", bufs=2))
    exp_pool = ctx.enter_context(tc.tile_pool(name="expp", bufs=2))
    osb_pool = ctx.enter_context(tc.tile_pool(name="osb", bufs=2))
    rec_pool = ctx.enter_context(tc.tile_pool(name="rec", bufs=4))
    ps_s = ctx.enter_context(tc.tile_pool(name="ps_s", bufs=3, space="PSUM"))
    ps_acc = ctx.enter_context(tc.tile_pool(name="ps_acc", bufs=1, space="PSUM"))

    identity = consts.tile([P, P], F32)
    make_identity(nc, identity[:])

    q_ap, k_ap, v_ap, out_ap = q[:], k[:], v[:], out[:]
    CH = 4  # tiles per dma/transpose chunk (= one q-block's worth)

    def load_transpose_chunk(r3, dst, c):
        """DMA 4 natural [128,128] tiles and PE-transpose them into dst."""
        nat = nat_pool.tile([P, CH, P], F32, tag="nat")
        nc.sync.dma_start(out=nat[:], in_=r3[:, c * CH : (c + 1) * CH, :])
        tp = ps_s.tile([P, CH, P], F32, tag="s")
        for u in range(CH):
            nc.tensor.transpose(tp[:, u, :], nat[:, u, :], identity[:])
        nc.vector.tensor_copy(dst[:, c * CH * P : (c + 1) * CH * P], tp[:])

    for n in range(H):
        # Per-head tensors; chunk c is loaded/transposed just before block c
        # (block b only reads K/V chunks <= b and Q chunk b), so the load +
        # transpose pipeline weaves into the matmul stream.
        kr3 = k_ap[n].rearrange("(j p) f -> p j f", p=P)
        qr3 = q_ap[n].rearrange("(j p) f -> p j f", p=P)
        vr3 = v_ap[n].rearrange("(j p) f -> p j f", p=P)
        kT = big_pool.tile([P, T], F32R, tag="kT")
        qT = big_pool.tile([P, T], F32R, tag="qT")
        v_aug = vpool.tile([P, NT, P + 1], BF16, tag="vaug")
        nc.vector.memset(v_aug[:, :, P : P + 1], 1.0)
        out_sb = osb_pool.tile([P, NT, P], F32, tag="osb")

        def load_chunks(c, kv=True):
            if kv:
                load_transpose_chunk(kr3, kT, c)
                # SWDGE casts fp32 -> bf16 in flight
                nc.gpsimd.dma_start(
                    out=v_aug[:, c * CH : (c + 1) * CH, 0:P],
                    in_=vr3[:, c * CH : (c + 1) * CH, :],
                )
            load_transpose_chunk(qr3, qT, c)

        def normalize_and_store(acc_sb, b):
            rec4 = rec_pool.tile([P, BLK], F32, tag="rec")
            nc.vector.reciprocal(rec4[:], acc_sb[:, :, P : P + 1])
            for ii in range(BLK):
                i = BLK * b + ii
                nc.vector.tensor_scalar_mul(
                    out_sb[:, i, :], acc_sb[:, ii, 0:P], rec4[:, ii : ii + 1]
                )
            nc.sync.dma_start(
                out=out_ap[n].rearrange("(i p) f -> p i f", p=P)[
                    :, BLK * b : BLK * (b + 1), :
                ],
                in_=out_sb[:, BLK * b : BLK * (b + 1), :],
            )

        # ---- main block loop over 512-wide q blocks ----
        # j-tiles are processed in pairs through 3 rotating 2-bank PSUM
        # score buffers: QK of pair g+2, exp of pair g+1, and AV of pair g
        # all run concurrently.  Chunk loads for block b+1 and the previous
        # block's normalize run mid-block, off the boundary critical path.
        pending = None
        for b in range(NBLK):
            if b == 0:
                if masked:
                    load_chunks(0)
                else:
                    for c in range(NBLK):
                        load_chunks(c, kv=True) if c == 0 else load_transpose_chunk(
                            kr3, kT, c
                        )
                        if c > 0:
                            nc.gpsimd.dma_start(
                                out=v_aug[:, c * CH : (c + 1) * CH, 0:P],
                                in_=vr3[:, c * CH : (c + 1) * CH, :],
                            )
            n_j = 4 * (b + 1) if masked else NT
            # Accumulators all share 2 PSUM banks at 256-fp32 stride. No
            # start=True anywhere (it would clear the whole bank's
            # has_written bits, corrupting the neighbour accumulator) —
            # instead zero the data once and let the first matmul either
            # overwrite (hw=0) or accumulate onto zero (stale hw=1).
            accs = ps_acc.tile([P, BLK, 256], F32, tag="acc")  # 2 PSUM banks
            nc.vector.memset(accs[:, :, 0 : P + 1], 0.0)
            for g0 in range(0, n_j, 2):
                if g0 == 2:
                    # mid-block: previous block's normalize + next block's
                    # chunk loads run here, clear of the boundary handoff
                    if pending is not None:
                        normalize_and_store(*pending)
                        pending = None
                    if b + 1 < NBLK:
                        load_chunks(b + 1, kv=masked)
                gsz = min(2, n_j - g0)
                scores = ps_s.tile([P, 2, 512], F32, tag="s")
                for r in range(gsz):
                    j = g0 + r
                    nc.tensor.matmul(
                        scores[:, r, :],
                        lhsT=kT[:, ts(j, P)],
                        rhs=qT[:, ts(b, 512)],
                        start=True,
                        stop=True,
                    )
                # diagonal pairs only need the causal span of columns
                col_lo = 0
                if masked and g0 - 4 * b >= 0:
                    col_lo = P * (g0 - 4 * b)
                expT = exp_pool.tile([P, 2, 512], BF16, tag="expT")
                nc.scalar.activation(
                    expT[:, 0:gsz, col_lo:512],
                    scores[:, 0:gsz, col_lo:512],
                    mybir.ActivationFunctionType.Exp,
                    scale=SCALE,
                )
                if masked:
                    # zero the upper-triangular (non-causal) band of any
                    # diagonal tile, post-exp, on the otherwise-idle gpsimd
                    for r in range(gsz):
                        ii = g0 + r - 4 * b
                        if 0 <= ii < BLK:
                            nc.gpsimd.affine_select(
                                out=expT[:, r, ts(ii, P)],
                                in_=expT[:, r, ts(ii, P)],
                                compare_op=mybir.AluOpType.is_ge,
                                fill=0.0,
                                base=0,
                                pattern=[[1, P]],
                                channel_multiplier=-1,
                            )
                for r in range(gsz):
                    j = g0 + r
                    for ii in range(BLK):
                        i = BLK * b + ii
                        if masked and j > i:
                            continue
                        nc.tensor.matmul(
                            accs[:, ii, 0 : P + 1],
                            lhsT=expT[:, r, ts(ii, P)],
                            rhs=v_aug[:, j, :],
                            start=False,
                            stop=(j == (i if masked else NT - 1)),
                            skip_group_check=True,
                        )
            # ---- evacuate accumulators; normalize is deferred ----
            # one strided copy frees the acc banks so the next block's AV
            # matmuls only wait on this copy, not the normalize sequence
            acc_sb = rec_pool.tile([P, BLK, P + 1], F32, tag="accsb")
            nc.vector.tensor_copy(acc_sb[:], accs[:, :, 0 : P + 1])
            pending = (acc_sb, b)
        if pending is not None:
            normalize_and_store(*pending)
            pending = None

    ctx.close()


_CACHE = {}


def _get_nc(masked: bool):
    key = bool(masked)
    if key not in _CACHE:
        _CACHE[key] = build(key)
    return _CACHE[key]


def _run(q, k, v, masked, **kwargs):
    nc = _get_nc(masked)
    q = np.ascontiguousarray(np.asarray(q, dtype=np.float32))
    k = np.ascontiguousarray(np.asarray(k, dtype=np.float32))
    v = np.ascontiguousarray(np.asarray(v, dtype=np.float32))
    in_maps = [
        {
            "q": q[c * H : (c + 1) * H],
            "k": k[c * H : (c + 1) * H],
            "v": v[c * H : (c + 1) * H],
        }
        for c in range(N_CORES)
    ]
    res = run_bass_kernel_spmd(nc, in_maps, core_ids=list(range(N_CORES)), **kwargs)
    outs = np.concatenate([r["out"] for r in res.results], axis=0)
    return outs, res


def kernel(q, k, v, masked):
    m = int(np.asarray(masked))
    outs, _ = _run(q, k, v, m != 0)
    return outs


if __name__ == "__main__":
    rng = np.random.default_rng(0)
    qq = rng.standard_normal((N, T, F), dtype=np.float32)
    kk = rng.standard_normal((N, T, F), dtype=np.float32)
    vv = rng.standard_normal((N, T, F), dtype=np.float32)
    o = kernel(qq, kk, vv, 1)
    print("out", o.shape, o.dtype, float(np.abs(o).mean()))


# revision 35
# speedup vs baseline: 1.0545x; 1.0545x over previous
"""Causal dot-product attention on 8 Trainium2 NeuronCores.

Problem: q,k,v [16, 2048, 128] fp32, causal softmax(q k^T / sqrt(128)) v.
Sharding: heads (N=16) split across 8 cores, 2 heads per core; no cross-core
communication.

Per-core kernel design (per head):
  - Q and K are transposed to [F, T] layout via PE transposes (the matmul
    contraction must sit on the partition dim), V stays natural [T, F] with an
    extra all-ones column appended (provides softmax row-sums for free via the
    attention matmul itself).
  - Scores are computed transposed, scoresT[s, q] = kT_j.T @ qT, one 512-wide
    q-block at a time, accumulated per 4 k-tiles into a [128, 2048] PSUM tile,
    masked on the diagonal band, exp'd on the scalar engine PSUM->SBUF.
  - out[q, f+1] accumulates expT_ij.T @ [v_j | 1] over j in PSUM; column 128 is
    the softmax denominator. Final normalize = per-partition reciprocal + mul.
"""

import numpy as np

import concourse.bass as bass
import concourse.mybir as mybir
import concourse.tile as tile
from concourse import bacc
from concourse.bass import ts
from concourse.bass_utils import run_bass_kernel_spmd
from concourse.masks import make_identity

N, T, F = 16, 2048, 128
N_CORES = 8
H = N // N_CORES  # heads per core
P = 128
NT = T // P  # 16 k/q tiles per head
BLK = 4  # q-tiles per block (512 q columns)
NBLK = NT // BLK
SCALE = 1.0 / float(np.sqrt(F))
NEG = -1.0e9

F32 = mybir.dt.float32
F32R = mybir.dt.float32r  # TF32-like PE mode: 1 cycle/row at N>=256 (fp32 is 4)
BF16 = mybir.dt.bfloat16


def build(masked: bool):
    nc = bacc.Bacc("TRN2", target_bir_lowering=False, debug=False, num_devices=N_CORES)
    q = nc.dram_tensor("q", [H, T, F], F32, kind="ExternalInput")
    k = nc.dram_tensor("k", [H, T, F], F32, kind="ExternalInput")
    v = nc.dram_tensor("v", [H, T, F], F32, kind="ExternalInput")
    out = nc.dram_tensor("out", [H, T, F], F32, kind="ExternalOutput")

    with tile.TileContext(nc) as tc:
        _attention(tc, out, q, k, v, masked)
    nc.compile()
    return nc


def _attention(tc, out, q, k, v, masked: bool):
    from contextlib import ExitStack

    nc = tc.nc
    ctx = ExitStack()
    consts = ctx.enter_context(tc.tile_pool(name="consts", bufs=1))
    nat_pool = ctx.enter_context(tc.tile_pool(name="nat", bufs=4))
    big_pool = ctx.enter_context(tc.tile_pool(name="big", bufs=2))
    vpool = ctx.enter_context(tc.tile_pool(name="vpool", bufs=2))
    exp_pool = ctx.enter_context(tc.tile_pool(name="expp", bufs=2))
    osb_pool = ctx.enter_context(tc.tile_pool(name="osb", bufs=2))
    rec_pool = ctx.enter_context(tc.tile_pool(name="rec", bufs=4))
    ps_s = ctx.enter_context(tc.tile_pool(name="ps_s", bufs=3, space="PSUM"))
    ps_acc = ctx.enter_context(tc.tile_pool(name="ps_acc", bufs=1, space="PSUM"))

    identity = consts.tile([P, P], F32)
    make_identity(nc, identity[:])

    q_ap, k_ap, v_ap, out_ap = q[:], k[:], v[:], out[:]
    CH = 4  # tiles per dma/transpose chunk (= one q-block's worth)

    def load_transpose_chunk(r3, dst, c):
        """DMA 4 natural [128,128] tiles and PE-transpose them into dst."""
        nat = nat_pool.tile([P, CH, P], F32, tag="nat")
        nc.sync.dma_start(out=nat[:], in_=r3[:, c * CH : (c + 1) * CH, :])
        tp = ps_s.tile([P, CH, P], F32, tag="s")
        for u in range(CH):
            nc.tensor.transpose(tp[:, u, :], nat[:, u, :], identity[:])
        nc.vector.tensor_copy(dst[:, c * CH * P : (c + 1) * CH * P], tp[:])

    for n in range(H):
        # Per-head tensors; chunk c is loaded/transposed just before block c
        # (block b only reads K/V chunks <= b and Q chunk b), so the load +
        # transpose pipeline weaves into the matmul stream.
        kr3 = k_ap[n].rearrange("(j p) f -> p j f", p=P)
        qr3 = q_ap[n].rearrange("(j p) f -> p j f", p=P)
        vr3 = v_ap[n].rearrange("(j p) f -> p j f", p=P)
        kT = big_pool.tile([P, T], F32R, tag="kT")
        qT = big_pool.tile([P, T], F32R, tag="qT")
        v_aug = vpool.tile([P, NT, P + 1], BF16, tag="vaug")
        nc.vector.memset(v_aug[:, :, P : P + 1], 1.0)
        out_sb = osb_pool.tile([P, NT, P], F32, tag="osb")

        def load_chunks(c, kv=True):
            if kv:
                load_transpose_chunk(kr3, kT, c)
                # SWDGE casts fp32 -> bf16 in flight
                nc.gpsimd.dma_start(
                    out=v_aug[:, c * CH : (c + 1) * CH, 0:P],
                    in_=vr3[:, c * CH : (c + 1) * CH, :],
                )
            load_transpose_chunk(qr3, qT, c)

        def normalize_and_store(acc_sb, b):
            rec4 = rec_pool.tile([P, BLK], F32, tag="rec")
            nc.vector.reciprocal(rec4[:], acc_sb[:, :, P : P + 1])
            for ii in range(BLK):
                i = BLK * b + ii
                nc.vector.tensor_scalar_mul(
                    out_sb[:, i, :], acc_sb[:, ii, 0:P], rec4[:, ii : ii + 1]
                )
            nc.sync.dma_start(
                out=out_ap[n].rearrange("(i p) f -> p i f", p=P)[
                    :, BLK * b : BLK * (b + 1), :
                ],
                in_=out_sb[:, BLK * b : BLK * (b + 1), :],
            )

        # ---- main block loop over 512-wide q blocks ----
        # j-tiles are processed in pairs through 3 rotating 2-bank PSUM
        # score buffers: QK of pair g+2, exp of pair g+1, and AV of pair g
        # all run concurrently.  Chunk loads for block b+1 and the previous
        # block's normalize run mid-block, off the boundary critical path.
        pending = None
        for b in range(NBLK):
            if b == 0:
                if masked:
                    load_chunks(0)
                else:
                    for c in range(NBLK):
                        load_chunks(c, kv=True) if c == 0 else load_transpose_chunk(
                            kr3, kT, c
                        )
                        if c > 0:
                            nc.gpsimd.dma_start(
                                out=v_aug[:, c * CH : (c + 1) * CH, 0:P],
                                in_=vr3[:, c * CH : (c + 1) * CH, :],
                            )
            n_j = 4 * (b + 1) if masked else NT
            # Accumulators all share 2 PSUM banks at 256-fp32 stride. No
            # start=True anywhere (it would clear the whole bank's
            # has_written bits, corrupting the neighbour accumulator) —
            # instead zero the data once and let the first matmul either
            # overwrite (hw=0) or accumulate onto zero (stale hw=1).
            accs = ps_acc.tile([P, BLK, 256], F32, tag="acc")  # 2 PSUM banks
            nc.vector.memset(accs[:, :, 0 : P + 1], 0.0)
            inject_at = max(2, (n_j // 2) & ~1)
            for g0 in range(0, n_j, 2):
                if g0 == inject_at:
                    # mid-block: previous block's normalize + next block's
                    # chunk loads run here, clear of the boundary handoff
                    if pending is not None:
                        normalize_and_store(*pending)
                        pending = None
                    if b + 1 < NBLK:
                        load_chunks(b + 1, kv=masked)
                gsz = min(2, n_j - g0)
                scores = ps_s.tile([P, 2, 512], F32, tag="s")
                for r in range(gsz):
                    j = g0 + r
                    nc.tensor.matmul(
                        scores[:, r, :],
                        lhsT=kT[:, ts(j, P)],
                        rhs=qT[:, ts(b, 512)],
                        start=True,
                        stop=True,
                    )
                # diagonal pairs only need the causal span of columns
                col_lo = 0
                if masked and g0 - 4 * b >= 0:
                    col_lo = P * (g0 - 4 * b)
                expT = exp_pool.tile([P, 2, 512], BF16, tag="expT")
                nc.scalar.activation(
                    expT[:, 0:gsz, col_lo:512],
                    scores[:, 0:gsz, col_lo:512],
                    mybir.ActivationFunctionType.Exp,
                    scale=SCALE,
                )
                if masked:
                    # zero the upper-triangular (non-causal) band of any
                    # diagonal tile, post-exp, on the otherwise-idle gpsimd
                    for r in range(gsz):
                        ii = g0 + r - 4 * b
                        if 0 <= ii < BLK:
                            nc.gpsimd.affine_select(
                                out=expT[:, r, ts(ii, P)],
                                in_=expT[:, r, ts(ii, P)],
                                compare_op=mybir.AluOpType.is_ge,
                                fill=0.0,
                                base=0,
                                pattern=[[1, P]],
                                channel_multiplier=-1,
                            )
                for r in range(gsz):
                    j = g0 + r
                    for ii in range(BLK):
                        i = BLK * b + ii
                        if masked and j > i:
                            continue
                        nc.tensor.matmul(
                            accs[:, ii, 0 : P + 1],
                            lhsT=expT[:, r, ts(ii, P)],
                            rhs=v_aug[:, j, :],
                            start=False,
                            stop=(j == (i if masked else NT - 1)),
                            skip_group_check=True,
                        )
            # ---- evacuate accumulators; normalize is deferred ----
            # one strided copy frees the acc banks so the next block's AV
            # matmuls only wait on this copy, not the normalize sequence
            acc_sb = rec_pool.tile([P, BLK, P + 1], F32, tag="accsb")
            nc.vector.tensor_copy(acc_sb[:], accs[:, :, 0 : P + 1])
            pending = (acc_sb, b)
        if pending is not None:
            normalize_and_store(*pending)
            pending = None

    ctx.close()


_CACHE = {}


def _get_nc(masked: bool):
    key = bool(masked)
    if key not in _CACHE:
        _CACHE[key] = build(key)
    return _CACHE[key]


def _run(q, k, v, masked, **kwargs):
    nc = _get_nc(masked)
    q = np.ascontiguousarray(np.asarray(q, dtype=np.float32))
    k = np.ascontiguousarray(np.asarray(k, dtype=np.float32))
    v = np.ascontiguousarray(np.asarray(v, dtype=np.float32))
    in_maps = [
        {
            "q": q[c * H : (c + 1) * H],
            "k": k[c * H : (c + 1) * H],
            "v": v[c * H : (c + 1) * H],
        }
        for c in range(N_CORES)
    ]
    res = run_bass_kernel_spmd(nc, in_maps, core_ids=list(range(N_CORES)), **kwargs)
    outs = np.concatenate([r["out"] for r in res.results], axis=0)
    return outs, res


def kernel(q, k, v, masked):
    m = int(np.asarray(masked))
    outs, _ = _run(q, k, v, m != 0)
    return outs


if __name__ == "__main__":
    rng = np.random.default_rng(0)
    qq = rng.standard_normal((N, T, F), dtype=np.float32)
    kk = rng.standard_normal((N, T, F), dtype=np.float32)
    vv = rng.standard_normal((N, T, F), dtype=np.float32)
    o = kernel(qq, kk, vv, 1)
    print("out", o.shape, o.dtype, float(np.abs(o).mean()))
